# revision 7
# baseline (speedup 1.0000x reference)
"""ClusterGCN (3-layer) + projection head on 8 trn2 NeuronCores.

Sharding: nodes partitioned 8 ways (6272 rows/core, padded table 50176).
Each core owns its node slice; the full activation table z is replicated in
every core's DRAM and refreshed between layers with an AllGather collective.
Edges are partitioned by target core, grouped per 128-target tile, and the
per-layer SpMM is computed as: dma_gather of source rows (int16 indices,
low/high table halves) -> one-hot matmul segment-sum into PSUM -> deg_inv
scale.  Dense matmuls run feature-major (weights stationary), activations
are kept transposed in SBUF across layers, and PE transposes produce the
node-major table written back to DRAM.
"""
import numpy as np

N = 50000
E = 800000
H = 256
P1 = 128
L = 3
NCORES = 8
OWN = 6272                # nodes per core (49 tiles x 128)
TBL = OWN * NCORES        # 50176
HALF = TBL // 2           # 25088 (int16 index limit per gather call)
NT = OWN // 128           # 49 target tiles per core
CALL_A = 1024             # idxs in first gather call of a part (8 blocks)


def _preprocess(node_x, edge_index):
    """Host-side integer preprocessing: edge grouping + paddings."""
    src = np.asarray(edge_index[0], np.int64)
    dst = np.asarray(edge_index[1], np.int64)
    loop = np.arange(N, dtype=np.int64)
    src = np.concatenate([src, loop])
    dst = np.concatenate([dst, loop])

    deg = np.bincount(dst, minlength=TBL).astype(np.float32)
    deginv = 1.0 / np.maximum(deg, 1.0)

    nx = np.full(TBL, -1, np.int64)
    nx[:N] = np.asarray(node_x, np.int64)

    # per (core, tile, half) edge lists
    per_core = []
    k2max = 1
    for c in range(NCORES):
        base = c * OWN
        sel = (dst >= base) & (dst < base + OWN)
        s_c, d_c = src[sel], dst[sel] - base
        tl = d_c // 128
        parts = []  # (srcs_local, colrel) per (tile, half)
        for t in range(NT):
            m = tl == t
            s_t, d_t = s_c[m], d_c[m] % 128
            lo = s_t < HALF
            parts.append((s_t[lo], d_t[lo]))
            parts.append((s_t[~lo] - HALF, d_t[~lo]))
            k2max = max(k2max, (len(s_t[lo]) + 127) // 128,
                        (len(s_t[~lo]) + 127) // 128)
        per_core.append(parts)
    k2 = max(k2max, 9)  # at least 9 so call B exists
    cap = k2 * 128

    idx16_all, colrel_all, deginv_all, onehot_all = [], [], [], []
    for c in range(NCORES):
        idx_cols, col_cols = [], []
        for (s_p, d_p) in per_core[c]:
            n_r = len(s_p)
            idxs = np.zeros(cap, np.int16)
            cols = np.full(cap, -1.0, np.float32)
            idxs[:n_r] = s_p.astype(np.int16)
            cols[:n_r] = d_p.astype(np.float32)
            # wrapped idx layout per call: j -> [j%16, j//16], replicated x8
            wr = []
            for st, sz in ((0, CALL_A), (CALL_A, cap - CALL_A)):
                w = idxs[st:st + sz].reshape(-1, 16).T  # [16, sz/16]
                wr.append(np.tile(w, (8, 1)))
            idx_cols.append(np.concatenate(wr, axis=1))   # [128, cap/16]
            col_cols.append(cols.reshape(-1, 128).T)      # [128, k2]
        idx16_all.append(np.concatenate(idx_cols, axis=1).astype(np.int16))
        colrel_all.append(np.concatenate(col_cols, axis=1).astype(np.float32))
        dv = deginv[c * OWN:(c + 1) * OWN]
        deginv_all.append(dv.reshape(NT, 128).T.copy())   # [128, 49]
        oh = np.zeros((6, OWN), np.float32)
        nx_c = nx[c * OWN:(c + 1) * OWN]
        for k in range(6):
            oh[k] = (nx_c == k).astype(np.float32)
        onehot_all.append(oh)
    return k2, idx16_all, colrel_all, deginv_all, onehot_all


def _build(k2, prelu_a):
    import concourse.bass as bass
    import concourse.bacc as bacc
    import concourse.mybir as mybir
    import concourse.tile as tile
    from concourse import library_config
    from concourse.masks import make_identity

    f32 = mybir.dt.float32
    cap = k2 * 128
    capw = cap // 16          # idx cols per part
    nblk = 2 * k2             # blocks per tile (low+high)
    NSL = 13                  # node slices: 12x512 + 1x128
    slices = [(s * 4, 4) for s in range(12)] + [(48, 1)]

    nc = bacc.Bacc("TRN2", target_bir_lowering=False, debug=False,
                   num_devices=NCORES, dynamic_dma_scratch_size=65536,
                   num_swdge_queues=4)

    wout = nc.dram_tensor("wout", [L, H, H], f32, kind="ExternalInput")
    wroot = nc.dram_tensor("wroot", [L, H, H], f32, kind="ExternalInput")
    bout = nc.dram_tensor("bout", [L * 2, 128, 1], f32, kind="ExternalInput")
    w1 = nc.dram_tensor("w1", [H, 128], f32, kind="ExternalInput")
    b1 = nc.dram_tensor("b1", [128, 1], f32, kind="ExternalInput")
    w2 = nc.dram_tensor("w2", [128, 128], f32, kind="ExternalInput")
    b2 = nc.dram_tensor("b2", [128, 1], f32, kind="ExternalInput")
    embed = nc.dram_tensor("embed", [6, H], f32, kind="ExternalInput")
    onehot = nc.dram_tensor("onehot", [6, OWN], f32, kind="ExternalInput")
    iota = nc.dram_tensor("iota", [128, 128], f32, kind="ExternalInput")
    deginv = nc.dram_tensor("deginv", [128, NT], f32, kind="ExternalInput")
    idx16 = nc.dram_tensor("idx16", [128, NT * 2 * capw], mybir.dt.int16,
                           kind="ExternalInput")
    colrel = nc.dram_tensor("colrel", [128, NT * nblk], f32,
                            kind="ExternalInput")
    z_out = nc.dram_tensor("z_out", [OWN, H], f32, kind="ExternalOutput")
    proj_out = nc.dram_tensor("proj_out", [OWN, 128], f32,
                              kind="ExternalOutput")

    z_table = nc.dram_tensor("z_table", [TBL, H], f32, kind="Internal",
                             addr_space="Shared")
    slab = nc.dram_tensor("slab", [OWN, H], f32, kind="Internal")

    with tile.TileContext(nc) as tc:
        with (
            tc.tile_pool(name="persist", bufs=1) as pp,
            tc.tile_pool(name="msg", bufs=3) as msgp,
            tc.tile_pool(name="work", bufs=2) as wkp,
            tc.tile_pool(name="aggt", bufs=1) as aggp,
            tc.tile_pool(name="ps_seg", bufs=2, space="PSUM") as ps_seg,
            tc.tile_pool(name="ps_t", bufs=4, space="PSUM") as ps_t,
            tc.tile_pool(name="ps_z", bufs=2, space="PSUM") as ps_z,
        ):
            nc.gpsimd.load_library(library_config.mlp)

            idx_sb = pp.tile([128, NT * 2 * capw], mybir.dt.int16, tag="idx")
            col_sb = pp.tile([128, NT * nblk], f32, tag="col")
            dv_sb = pp.tile([128, NT], f32, tag="dv")
            io_sb = pp.tile([128, 128], f32, tag="iota")
            id_sb = pp.tile([128, 128], f32, tag="ident")
            wo_sb = pp.tile([128, L * 2 * H], f32, tag="wo")
            wr_sb = pp.tile([128, L * 2 * H], f32, tag="wr")
            bo_sb = pp.tile([128, L * 2], f32, tag="bo")
            w1_sb = pp.tile([128, 2 * 128], f32, tag="w1")
            b1_sb = pp.tile([128, 1], f32, tag="b1")
            w2_sb = pp.tile([128, 128], f32, tag="w2")
            b2_sb = pp.tile([128, 1], f32, tag="b2")
            em_sb = pp.tile([6, H], f32, tag="em")
            zt0 = pp.tile([128, OWN], f32, tag="zt0")
            zt1 = pp.tile([128, OWN], f32, tag="zt1")
            zt = [zt0, zt1]

            nc.sync.dma_start(idx_sb[:], idx16[:, :])
            nc.sync.dma_start(col_sb[:], colrel[:, :])
            nc.sync.dma_start(dv_sb[:], deginv[:, :])
            nc.sync.dma_start(io_sb[:], iota[:, :])
            nc.sync.dma_start(em_sb[:], embed[:, :])
            make_identity(nc, id_sb[:])
            for l in range(L):
                for f in range(2):
                    nc.sync.dma_start(
                        wo_sb[:, (l * 2 + f) * H:(l * 2 + f + 1) * H],
                        wout[l, f * 128:(f + 1) * 128, :])
                    nc.sync.dma_start(
                        wr_sb[:, (l * 2 + f) * H:(l * 2 + f + 1) * H],
                        wroot[l, f * 128:(f + 1) * 128, :])
                    nc.sync.dma_start(
                        bo_sb[:, l * 2 + f:l * 2 + f + 1],
                        bout[l * 2 + f, :, :])
            for f in range(2):
                nc.sync.dma_start(w1_sb[:, f * 128:(f + 1) * 128],
                                  w1[f * 128:(f + 1) * 128, :])
            nc.sync.dma_start(b1_sb[:], b1[:, :])
            nc.sync.dma_start(w2_sb[:], w2[:, :])
            nc.sync.dma_start(b2_sb[:], b2[:, :])

            with tc.tile_pool(name="z0", bufs=1) as z0p:
                for si, (t0, w) in enumerate(slices):
                    oh_c = z0p.tile([6, 512], f32, tag="ohc")
                    nc.sync.dma_start(oh_c[:, :w * 128],
                                      onehot[:, t0 * 128:t0 * 128 + w * 128])
                    for r in range(w):
                        ps = ps_seg.tile([128, H], f32, tag="seg")
                        nc.tensor.matmul(ps[:],
                                         oh_c[:, r * 128:(r + 1) * 128],
                                         em_sb[:], start=True, stop=True)
                        zrow = z0p.tile([128, H], f32, tag="z0row")
                        nc.scalar.activation(zrow[:], ps[:],
                                             bass.mybir.ActivationFunctionType.Copy)
                        nc.sync.dma_start(
                            slab[(t0 + r) * 128:(t0 + r + 1) * 128, :],
                            zrow[:])
                    for f in range(2):
                        psz = ps_z.tile([128, 512], f32, tag="pz")
                        nc.tensor.matmul(
                            psz[:, :w * 128],
                            em_sb[:, f * 128:(f + 1) * 128],
                            oh_c[:, :w * 128], start=True, stop=True)
                        nc.scalar.activation(
                            zt[f][:, t0 * 128:t0 * 128 + w * 128],
                            psz[:, :w * 128],
                            bass.mybir.ActivationFunctionType.Copy)
            nc.gpsimd.collective_compute(
                "AllGather", bass.mybir.AluOpType.bypass,
                replica_groups=[list(range(NCORES))],
                ins=[slab.ap()], outs=[z_table.ap()])

            for l in range(L):
                for s, (t0, w) in enumerate(slices):
                    agT = [aggp.tile([128, 512], f32, tag=f"agT{f}",
                                     name=f"agT{f}_{l}_{s}")
                           for f in range(2)]
                    for r in range(w):
                        t = t0 + r
                        ps = ps_seg.tile([128, H], f32, tag="seg")
                        first = True
                        for half in range(2):
                            part = t * 2 + half
                            m = msgp.tile([128, k2, H], f32, tag="msg")
                            src_ap = (z_table[0:HALF, :] if half == 0
                                      else z_table[HALF:TBL, :])
                            ioff = part * capw
                            nc.gpsimd.dma_gather(
                                out_ap=m[:, 0:8, :], in_ap=src_ap,
                                idxs_ap=idx_sb[:, ioff:ioff + 64],
                                num_idxs=CALL_A, num_idxs_reg=CALL_A,
                                elem_size=H, queue_num=(2 * part) % 4)
                            nc.gpsimd.dma_gather(
                                out_ap=m[:, 8:k2, :], in_ap=src_ap,
                                idxs_ap=idx_sb[:, ioff + 64:ioff + capw],
                                num_idxs=cap - CALL_A,
                                num_idxs_reg=cap - CALL_A,
                                elem_size=H, queue_num=(2 * part + 1) % 4)
                            for b in range(k2):
                                blk = t * nblk + half * k2 + b
                                S = wkp.tile([128, 128], f32, tag="S")
                                nc.vector.tensor_scalar(
                                    out=S[:], in0=io_sb[:],
                                    scalar1=col_sb[:, blk:blk + 1],
                                    scalar2=None,
                                    op0=bass.mybir.AluOpType.is_equal)
                                nc.tensor.matmul(
                                    ps[:], S[:], m[:, b, :],
                                    start=first, stop=(half == 1 and
                                                       b == k2 - 1))
                                first = False
                        agg = wkp.tile([128, H], f32, tag="agg")
                        nc.vector.tensor_scalar_mul(
                            agg[:], ps[:], dv_sb[:, t:t + 1])
                        for f in range(2):
                            pt = ps_t.tile([128, 128], f32, tag="pt")
                            nc.tensor.transpose(
                                pt[:], agg[:, f * 128:(f + 1) * 128],
                                id_sb[:])
                            nc.vector.tensor_copy(
                                agT[f][:, r * 128:(r + 1) * 128], pt[:])
                    # dense matmuls: both psums fully read zt BEFORE the
                    # in-place ACT writes below overwrite this slice of zt
                    pzs = []
                    for f in range(2):   # output feature chunk
                        pz = ps_z.tile([128, 512], f32, tag="pz")
                        for fi in range(2):
                            nc.tensor.matmul(
                                pz[:, :w * 128],
                                wo_sb[:, (l * 2 + fi) * H + f * 128:
                                      (l * 2 + fi) * H + (f + 1) * 128],
                                agT[fi][:, :w * 128],
                                start=(fi == 0), stop=False)
                        for fi in range(2):
                            nc.tensor.matmul(
                                pz[:, :w * 128],
                                wr_sb[:, (l * 2 + fi) * H + f * 128:
                                      (l * 2 + fi) * H + (f + 1) * 128],
                                zt[fi][:, t0 * 128:t0 * 128 + w * 128],
                                start=False, stop=(fi == 1))
                        pzs.append(pz)
                    for f in range(2):
                        nc.scalar.activation(
                            zt[f][:, t0 * 128:t0 * 128 + w * 128],
                            pzs[f][:, :w * 128],
                            bass.mybir.ActivationFunctionType.Relu,
                            bias=bo_sb[:, l * 2 + f:l * 2 + f + 1])
                    # node-major z_new tiles -> slab / z_out
                    for r in range(w):
                        t = t0 + r
                        zrow = wkp.tile([128, H], f32, tag="zrow")
                        for f in range(2):
                            pt = ps_t.tile([128, 128], f32, tag="pt")
                            nc.tensor.transpose(
                                pt[:], zt[f][:, t * 128:(t + 1) * 128],
                                id_sb[:])
                            nc.vector.tensor_copy(
                                zrow[:, f * 128:(f + 1) * 128], pt[:])
                        if l < L - 1:
                            nc.sync.dma_start(
                                slab[t * 128:(t + 1) * 128, :], zrow[:])
                        else:
                            nc.sync.dma_start(
                                z_out[t * 128:(t + 1) * 128, :], zrow[:])
                if l < L - 1:
                    nc.gpsimd.collective_compute(
                        "AllGather", bass.mybir.AluOpType.bypass,
                        replica_groups=[list(range(NCORES))],
                        ins=[slab.ap()], outs=[z_table.ap()])

            # head: h = prelu(z @ W1 + b1); proj = h @ W2 + b2
            ht = pp.tile([128, OWN], f32, tag="ht")
            for s, (t0, w) in enumerate(slices):
                ph = ps_z.tile([128, 512], f32, tag="pz")
                for fi in range(2):
                    nc.tensor.matmul(
                        ph[:, :w * 128],
                        w1_sb[:, fi * 128:(fi + 1) * 128],
                        zt[fi][:, t0 * 128:t0 * 128 + w * 128],
                        start=(fi == 0), stop=(fi == 1))
                pos = wkp.tile([128, 512], f32, tag="hpos")
                nc.vector.tensor_scalar(
                    out=pos[:, :w * 128], in0=ph[:, :w * 128],
                    scalar1=b1_sb[:, 0:1], scalar2=0.0,
                    op0=bass.mybir.AluOpType.add,
                    op1=bass.mybir.AluOpType.max)
                nc.vector.tensor_scalar(
                    out=ht[:, t0 * 128:t0 * 128 + w * 128],
                    in0=ph[:, :w * 128],
                    scalar1=b1_sb[:, 0:1], scalar2=0.0,
                    op0=bass.mybir.AluOpType.add,
                    op1=bass.mybir.AluOpType.min)
                nc.vector.tensor_scalar(
                    out=ht[:, t0 * 128:t0 * 128 + w * 128],
                    in0=ht[:, t0 * 128:t0 * 128 + w * 128],
                    scalar1=float(prelu_a), scalar2=None,
                    op0=bass.mybir.AluOpType.mult)
                nc.vector.tensor_add(
                    ht[:, t0 * 128:t0 * 128 + w * 128],
                    ht[:, t0 * 128:t0 * 128 + w * 128],
                    pos[:, :w * 128])
            for s, (t0, w) in enumerate(slices):
                pp2 = ps_z.tile([128, 512], f32, tag="pz")
                nc.tensor.matmul(pp2[:, :w * 128], w2_sb[:],
                                 ht[:, t0 * 128:t0 * 128 + w * 128],
                                 start=True, stop=True)
                pjt = wkp.tile([128, 512], f32, tag="pjt")
                nc.vector.tensor_scalar_add(pjt[:, :w * 128],
                                            pp2[:, :w * 128],
                                            b2_sb[:, 0:1])
                for r in range(w):
                    t = t0 + r
                    pt = ps_t.tile([128, 128], f32, tag="pt")
                    nc.tensor.transpose(
                        pt[:], pjt[:, r * 128:(r + 1) * 128], id_sb[:])
                    prow = wkp.tile([128, 128], f32, tag="prow")
                    nc.vector.tensor_copy(prow[:], pt[:])
                    nc.sync.dma_start(
                        proj_out[t * 128:(t + 1) * 128, :], prow[:])
    nc.compile()
    return nc


def kernel(node_x, edge_type, edge_index, node_embed, W_out, b_out, W_root,
           W1, b1, prelu_a, W2, b2):
    from concourse.bass_utils import run_bass_kernel_spmd

    k2, idx16_all, colrel_all, deginv_all, onehot_all = _preprocess(
        node_x, edge_index)
    nc = _build(k2, float(np.asarray(prelu_a)))

    iota_mat = np.tile(np.arange(128, dtype=np.float32), (128, 1)).copy()
    common = dict(
        wout=np.asarray(W_out, np.float32),
        wroot=np.asarray(W_root, np.float32),
        bout=np.asarray(b_out, np.float32).reshape(L, 2, 128).reshape(L * 2, 128, 1),
        w1=np.asarray(W1, np.float32), b1=np.asarray(b1, np.float32).reshape(128, 1),
        w2=np.asarray(W2, np.float32), b2=np.asarray(b2, np.float32).reshape(128, 1),
        embed=np.asarray(node_embed, np.float32), iota=iota_mat,
    )
    in_maps = []
    for c in range(NCORES):
        m = dict(common)
        m["onehot"] = onehot_all[c]
        m["deginv"] = deginv_all[c]
        m["idx16"] = idx16_all[c]
        m["colrel"] = colrel_all[c]
        in_maps.append(m)

    res = run_bass_kernel_spmd(nc, in_maps, core_ids=list(range(NCORES)))
    z = np.concatenate([r["z_out"] for r in res.results], axis=0)[:N]
    proj = np.concatenate([r["proj_out"] for r in res.results], axis=0)[:N]
    return (z, proj)


# revision 9
# speedup vs baseline: 1.0342x; 1.0342x over previous
"""ClusterGCN (3-layer) + projection head on 8 trn2 NeuronCores.

Sharding: nodes partitioned 8 ways (6272 rows/core, padded table 50176).
Each core owns its node slice; the full activation table z is replicated in
every core's DRAM and refreshed between layers with an AllGather collective.
Edges are partitioned by target core, grouped per 128-target tile, and the
per-layer SpMM is computed as: dma_gather of source rows (int16 indices,
low/high table halves) -> one-hot matmul segment-sum into PSUM -> deg_inv
scale.  Dense matmuls run feature-major (weights stationary), activations
are kept transposed in SBUF across layers, and PE transposes produce the
node-major table written back to DRAM.
"""
import numpy as np

N = 50000
E = 800000
H = 256
P1 = 128
L = 3
NCORES = 8
OWN = 6272                # nodes per core (49 tiles x 128)
TBL = OWN * NCORES        # 50176
HALF = TBL // 2           # 25088 (int16 index limit per gather call)
NT = OWN // 128           # 49 target tiles per core
CALL_A = 1024             # idxs in first gather call of a part (8 blocks)


def _preprocess(node_x, edge_index):
    """Host-side integer preprocessing: edge grouping + paddings."""
    src = np.asarray(edge_index[0], np.int64)
    dst = np.asarray(edge_index[1], np.int64)
    loop = np.arange(N, dtype=np.int64)
    src = np.concatenate([src, loop])
    dst = np.concatenate([dst, loop])

    deg = np.bincount(dst, minlength=TBL).astype(np.float32)
    deginv = 1.0 / np.maximum(deg, 1.0)

    nx = np.full(TBL, -1, np.int64)
    nx[:N] = np.asarray(node_x, np.int64)

    # per (core, tile, half) edge lists
    per_core = []
    k2max = 1
    for c in range(NCORES):
        base = c * OWN
        sel = (dst >= base) & (dst < base + OWN)
        s_c, d_c = src[sel], dst[sel] - base
        tl = d_c // 128
        parts = []  # (srcs_local, colrel) per (tile, half)
        for t in range(NT):
            m = tl == t
            s_t, d_t = s_c[m], d_c[m] % 128
            lo = s_t < HALF
            parts.append((s_t[lo], d_t[lo]))
            parts.append((s_t[~lo] - HALF, d_t[~lo]))
            k2max = max(k2max, (len(s_t[lo]) + 127) // 128,
                        (len(s_t[~lo]) + 127) // 128)
        per_core.append(parts)
    k2 = max(k2max, 9)  # at least 9 so call B exists
    cap = k2 * 128

    idx16_all, colrel_all, deginv_all, onehot_all = [], [], [], []
    for c in range(NCORES):
        idx_cols, col_cols = [], []
        for (s_p, d_p) in per_core[c]:
            n_r = len(s_p)
            idxs = np.zeros(cap, np.int16)
            cols = np.full(cap, -1.0, np.float32)
            idxs[:n_r] = s_p.astype(np.int16)
            cols[:n_r] = d_p.astype(np.float32)
            # wrapped idx layout per call: j -> [j%16, j//16], replicated x8
            wr = []
            for st, sz in ((0, CALL_A), (CALL_A, cap - CALL_A)):
                w = idxs[st:st + sz].reshape(-1, 16).T  # [16, sz/16]
                wr.append(np.tile(w, (8, 1)))
            idx_cols.append(np.concatenate(wr, axis=1))   # [128, cap/16]
            col_cols.append(cols.reshape(-1, 128).T)      # [128, k2]
        idx16_all.append(np.concatenate(idx_cols, axis=1).astype(np.int16))
        colrel_all.append(np.concatenate(col_cols, axis=1).astype(np.float32))
        dv = deginv[c * OWN:(c + 1) * OWN]
        deginv_all.append(dv.reshape(NT, 128).T.copy())   # [128, 49]
        oh = np.zeros((6, OWN), np.float32)
        nx_c = nx[c * OWN:(c + 1) * OWN]
        for k in range(6):
            oh[k] = (nx_c == k).astype(np.float32)
        onehot_all.append(oh)
    return k2, idx16_all, colrel_all, deginv_all, onehot_all


def _build(k2, prelu_a):
    import concourse.bass as bass
    import concourse.bacc as bacc
    import concourse.mybir as mybir
    import concourse.tile as tile
    from concourse import library_config
    from concourse.masks import make_identity

    f32 = mybir.dt.float32
    cap = k2 * 128
    capw = cap // 16          # idx cols per part
    nblk = 2 * k2             # blocks per tile (low+high)
    NSL = 13                  # node slices: 12x512 + 1x128
    slices = [(s * 4, 4) for s in range(12)] + [(48, 1)]

    nc = bacc.Bacc("TRN2", target_bir_lowering=False, debug=False,
                   num_devices=NCORES, dynamic_dma_scratch_size=65536,
                   num_swdge_queues=4)

    wout = nc.dram_tensor("wout", [L, H, H], f32, kind="ExternalInput")
    wroot = nc.dram_tensor("wroot", [L, H, H], f32, kind="ExternalInput")
    bout = nc.dram_tensor("bout", [L * 2, 128, 1], f32, kind="ExternalInput")
    w1 = nc.dram_tensor("w1", [H, 128], f32, kind="ExternalInput")
    b1 = nc.dram_tensor("b1", [128, 1], f32, kind="ExternalInput")
    w2 = nc.dram_tensor("w2", [128, 128], f32, kind="ExternalInput")
    b2 = nc.dram_tensor("b2", [128, 1], f32, kind="ExternalInput")
    embed = nc.dram_tensor("embed", [6, H], f32, kind="ExternalInput")
    onehot = nc.dram_tensor("onehot", [6, OWN], f32, kind="ExternalInput")
    iota = nc.dram_tensor("iota", [128, 128], f32, kind="ExternalInput")
    deginv = nc.dram_tensor("deginv", [128, NT], f32, kind="ExternalInput")
    idx16 = nc.dram_tensor("idx16", [128, NT * 2 * capw], mybir.dt.int16,
                           kind="ExternalInput")
    colrel = nc.dram_tensor("colrel", [128, NT * nblk], f32,
                            kind="ExternalInput")
    z_out = nc.dram_tensor("z_out", [OWN, H], f32, kind="ExternalOutput")
    proj_out = nc.dram_tensor("proj_out", [OWN, 128], f32,
                              kind="ExternalOutput")

    z_table = nc.dram_tensor("z_table", [TBL, H], f32, kind="Internal",
                             addr_space="Shared")
    slab = nc.dram_tensor("slab", [OWN, H], f32, kind="Internal")

    with tile.TileContext(nc) as tc:
        with (
            tc.tile_pool(name="persist", bufs=1) as pp,
            tc.tile_pool(name="msg", bufs=3) as msgp,
            tc.tile_pool(name="work", bufs=2) as wkp,
            tc.tile_pool(name="aggt", bufs=1) as aggp,
            tc.tile_pool(name="ps_seg", bufs=2, space="PSUM") as ps_seg,
            tc.tile_pool(name="ps_t", bufs=4, space="PSUM") as ps_t,
            tc.tile_pool(name="ps_z", bufs=2, space="PSUM") as ps_z,
        ):
            nc.gpsimd.load_library(library_config.mlp)

            idx_sb = pp.tile([128, NT * 2 * capw], mybir.dt.int16, tag="idx")
            col_sb = pp.tile([128, NT * nblk], f32, tag="col")
            dv_sb = pp.tile([128, NT], f32, tag="dv")
            io_sb = pp.tile([128, 128], f32, tag="iota")
            id_sb = pp.tile([128, 128], f32, tag="ident")
            wo_sb = pp.tile([128, L * 2 * H], f32, tag="wo")
            wr_sb = pp.tile([128, L * 2 * H], f32, tag="wr")
            bo_sb = pp.tile([128, L * 2], f32, tag="bo")
            w1_sb = pp.tile([128, 2 * 128], f32, tag="w1")
            b1_sb = pp.tile([128, 1], f32, tag="b1")
            w2_sb = pp.tile([128, 128], f32, tag="w2")
            b2_sb = pp.tile([128, 1], f32, tag="b2")
            em_sb = pp.tile([6, H], f32, tag="em")
            zt0 = pp.tile([128, OWN], f32, tag="zt0")
            zt1 = pp.tile([128, OWN], f32, tag="zt1")
            zt = [zt0, zt1]

            nc.sync.dma_start(idx_sb[:], idx16[:, :])
            nc.sync.dma_start(col_sb[:], colrel[:, :])
            nc.sync.dma_start(dv_sb[:], deginv[:, :])
            nc.sync.dma_start(io_sb[:], iota[:, :])
            nc.sync.dma_start(em_sb[:], embed[:, :])
            make_identity(nc, id_sb[:])
            for l in range(L):
                for f in range(2):
                    nc.sync.dma_start(
                        wo_sb[:, (l * 2 + f) * H:(l * 2 + f + 1) * H],
                        wout[l, f * 128:(f + 1) * 128, :])
                    nc.sync.dma_start(
                        wr_sb[:, (l * 2 + f) * H:(l * 2 + f + 1) * H],
                        wroot[l, f * 128:(f + 1) * 128, :])
                    nc.sync.dma_start(
                        bo_sb[:, l * 2 + f:l * 2 + f + 1],
                        bout[l * 2 + f, :, :])
            for f in range(2):
                nc.sync.dma_start(w1_sb[:, f * 128:(f + 1) * 128],
                                  w1[f * 128:(f + 1) * 128, :])
            nc.sync.dma_start(b1_sb[:], b1[:, :])
            nc.sync.dma_start(w2_sb[:], w2[:, :])
            nc.sync.dma_start(b2_sb[:], b2[:, :])

            with tc.tile_pool(name="z0", bufs=1) as z0p:
                for si, (t0, w) in enumerate(slices):
                    oh_c = z0p.tile([6, 512], f32, tag="ohc")
                    nc.sync.dma_start(oh_c[:, :w * 128],
                                      onehot[:, t0 * 128:t0 * 128 + w * 128])
                    for r in range(w):
                        ps = ps_seg.tile([128, H], f32, tag="seg")
                        nc.tensor.matmul(ps[:],
                                         oh_c[:, r * 128:(r + 1) * 128],
                                         em_sb[:], start=True, stop=True)
                        zrow = z0p.tile([128, H], f32, tag="z0row")
                        nc.scalar.activation(zrow[:], ps[:],
                                             bass.mybir.ActivationFunctionType.Copy)
                        nc.sync.dma_start(
                            slab[(t0 + r) * 128:(t0 + r + 1) * 128, :],
                            zrow[:])
                    for f in range(2):
                        psz = ps_z.tile([128, 512], f32, tag="pz")
                        nc.tensor.matmul(
                            psz[:, :w * 128],
                            em_sb[:, f * 128:(f + 1) * 128],
                            oh_c[:, :w * 128], start=True, stop=True)
                        nc.scalar.activation(
                            zt[f][:, t0 * 128:t0 * 128 + w * 128],
                            psz[:, :w * 128],
                            bass.mybir.ActivationFunctionType.Copy)
            nc.gpsimd.collective_compute(
                "AllGather", bass.mybir.AluOpType.bypass,
                replica_groups=[list(range(NCORES))],
                ins=[slab.ap()], outs=[z_table.ap()])

            for l in range(L):
                for s, (t0, w) in enumerate(slices):
                    agT = [aggp.tile([128, 512], f32, tag=f"agT{f}",
                                     name=f"agT{f}_{l}_{s}")
                           for f in range(2)]
                    for r in range(w):
                        t = t0 + r
                        ps = ps_seg.tile([128, H], f32, tag="seg")
                        first = True
                        for half in range(2):
                            part = t * 2 + half
                            m = msgp.tile([128, k2, H], f32, tag="msg")
                            src_ap = (z_table[0:HALF, :] if half == 0
                                      else z_table[HALF:TBL, :])
                            ioff = part * capw
                            nc.gpsimd.dma_gather(
                                out_ap=m[:, 0:8, :], in_ap=src_ap,
                                idxs_ap=idx_sb[:, ioff:ioff + 64],
                                num_idxs=CALL_A, num_idxs_reg=CALL_A,
                                elem_size=H, queue_num=(2 * part) % 4)
                            nc.gpsimd.dma_gather(
                                out_ap=m[:, 8:k2, :], in_ap=src_ap,
                                idxs_ap=idx_sb[:, ioff + 64:ioff + capw],
                                num_idxs=cap - CALL_A,
                                num_idxs_reg=cap - CALL_A,
                                elem_size=H, queue_num=(2 * part + 1) % 4)
                            for b in range(k2):
                                blk = t * nblk + half * k2 + b
                                S = wkp.tile([128, 128], f32, tag="S", bufs=4)
                                nc.vector.tensor_scalar(
                                    out=S[:], in0=io_sb[:],
                                    scalar1=col_sb[:, blk:blk + 1],
                                    scalar2=None,
                                    op0=bass.mybir.AluOpType.is_equal)
                                nc.tensor.matmul(
                                    ps[:], S[:], m[:, b, :],
                                    start=first, stop=(half == 1 and
                                                       b == k2 - 1))
                                first = False
                        agg = wkp.tile([128, H], f32, tag="agg")
                        nc.scalar.activation(
                            agg[:], ps[:],
                            bass.mybir.ActivationFunctionType.Copy,
                            scale=dv_sb[:, t:t + 1])
                        for f in range(2):
                            pt = ps_t.tile([128, 128], f32, tag="pt")
                            nc.tensor.transpose(
                                pt[:], agg[:, f * 128:(f + 1) * 128],
                                id_sb[:])
                            nc.vector.tensor_copy(
                                agT[f][:, r * 128:(r + 1) * 128], pt[:])
                    # dense matmuls: both psums fully read zt BEFORE the
                    # in-place ACT writes below overwrite this slice of zt
                    pzs = []
                    for f in range(2):   # output feature chunk
                        pz = ps_z.tile([128, 512], f32, tag="pz")
                        for fi in range(2):
                            nc.tensor.matmul(
                                pz[:, :w * 128],
                                wo_sb[:, (l * 2 + fi) * H + f * 128:
                                      (l * 2 + fi) * H + (f + 1) * 128],
                                agT[fi][:, :w * 128],
                                start=(fi == 0), stop=False)
                        for fi in range(2):
                            nc.tensor.matmul(
                                pz[:, :w * 128],
                                wr_sb[:, (l * 2 + fi) * H + f * 128:
                                      (l * 2 + fi) * H + (f + 1) * 128],
                                zt[fi][:, t0 * 128:t0 * 128 + w * 128],
                                start=False, stop=(fi == 1))
                        pzs.append(pz)
                    for f in range(2):
                        nc.scalar.activation(
                            zt[f][:, t0 * 128:t0 * 128 + w * 128],
                            pzs[f][:, :w * 128],
                            bass.mybir.ActivationFunctionType.Relu,
                            bias=bo_sb[:, l * 2 + f:l * 2 + f + 1])
                    # node-major z_new tiles -> slab / z_out
                    for r in range(w):
                        t = t0 + r
                        zrow = wkp.tile([128, H], f32, tag="zrow")
                        for f in range(2):
                            pt = ps_t.tile([128, 128], f32, tag="pt")
                            nc.tensor.transpose(
                                pt[:], zt[f][:, t * 128:(t + 1) * 128],
                                id_sb[:])
                            nc.vector.tensor_copy(
                                zrow[:, f * 128:(f + 1) * 128], pt[:])
                        if l < L - 1:
                            nc.sync.dma_start(
                                slab[t * 128:(t + 1) * 128, :], zrow[:])
                        else:
                            nc.sync.dma_start(
                                z_out[t * 128:(t + 1) * 128, :], zrow[:])
                if l < L - 1:
                    nc.gpsimd.collective_compute(
                        "AllGather", bass.mybir.AluOpType.bypass,
                        replica_groups=[list(range(NCORES))],
                        ins=[slab.ap()], outs=[z_table.ap()])

            # head: h = prelu(z @ W1 + b1); proj = h @ W2 + b2
            ht = pp.tile([128, OWN], f32, tag="ht")
            for s, (t0, w) in enumerate(slices):
                ph = ps_z.tile([128, 512], f32, tag="pz")
                for fi in range(2):
                    nc.tensor.matmul(
                        ph[:, :w * 128],
                        w1_sb[:, fi * 128:(fi + 1) * 128],
                        zt[fi][:, t0 * 128:t0 * 128 + w * 128],
                        start=(fi == 0), stop=(fi == 1))
                pos = wkp.tile([128, 512], f32, tag="hpos", bufs=1)
                nc.vector.tensor_scalar(
                    out=pos[:, :w * 128], in0=ph[:, :w * 128],
                    scalar1=b1_sb[:, 0:1], scalar2=0.0,
                    op0=bass.mybir.AluOpType.add,
                    op1=bass.mybir.AluOpType.max)
                nc.vector.tensor_scalar(
                    out=ht[:, t0 * 128:t0 * 128 + w * 128],
                    in0=ph[:, :w * 128],
                    scalar1=b1_sb[:, 0:1], scalar2=0.0,
                    op0=bass.mybir.AluOpType.add,
                    op1=bass.mybir.AluOpType.min)
                nc.vector.tensor_scalar(
                    out=ht[:, t0 * 128:t0 * 128 + w * 128],
                    in0=ht[:, t0 * 128:t0 * 128 + w * 128],
                    scalar1=float(prelu_a), scalar2=None,
                    op0=bass.mybir.AluOpType.mult)
                nc.vector.tensor_add(
                    ht[:, t0 * 128:t0 * 128 + w * 128],
                    ht[:, t0 * 128:t0 * 128 + w * 128],
                    pos[:, :w * 128])
            for s, (t0, w) in enumerate(slices):
                pp2 = ps_z.tile([128, 512], f32, tag="pz")
                nc.tensor.matmul(pp2[:, :w * 128], w2_sb[:],
                                 ht[:, t0 * 128:t0 * 128 + w * 128],
                                 start=True, stop=True)
                pjt = wkp.tile([128, 512], f32, tag="pjt", bufs=1)
                nc.vector.tensor_scalar_add(pjt[:, :w * 128],
                                            pp2[:, :w * 128],
                                            b2_sb[:, 0:1])
                for r in range(w):
                    t = t0 + r
                    pt = ps_t.tile([128, 128], f32, tag="pt")
                    nc.tensor.transpose(
                        pt[:], pjt[:, r * 128:(r + 1) * 128], id_sb[:])
                    prow = wkp.tile([128, 128], f32, tag="prow")
                    nc.vector.tensor_copy(prow[:], pt[:])
                    nc.sync.dma_start(
                        proj_out[t * 128:(t + 1) * 128, :], prow[:])
    nc.compile()
    return nc


def kernel(node_x, edge_type, edge_index, node_embed, W_out, b_out, W_root,
           W1, b1, prelu_a, W2, b2):
    from concourse.bass_utils import run_bass_kernel_spmd

    k2, idx16_all, colrel_all, deginv_all, onehot_all = _preprocess(
        node_x, edge_index)
    nc = _build(k2, float(np.asarray(prelu_a)))

    iota_mat = np.tile(np.arange(128, dtype=np.float32), (128, 1)).copy()
    common = dict(
        wout=np.asarray(W_out, np.float32),
        wroot=np.asarray(W_root, np.float32),
        bout=np.asarray(b_out, np.float32).reshape(L, 2, 128).reshape(L * 2, 128, 1),
        w1=np.asarray(W1, np.float32), b1=np.asarray(b1, np.float32).reshape(128, 1),
        w2=np.asarray(W2, np.float32), b2=np.asarray(b2, np.float32).reshape(128, 1),
        embed=np.asarray(node_embed, np.float32), iota=iota_mat,
    )
    in_maps = []
    for c in range(NCORES):
        m = dict(common)
        m["onehot"] = onehot_all[c]
        m["deginv"] = deginv_all[c]
        m["idx16"] = idx16_all[c]
        m["colrel"] = colrel_all[c]
        in_maps.append(m)

    res = run_bass_kernel_spmd(nc, in_maps, core_ids=list(range(NCORES)))
    z = np.concatenate([r["z_out"] for r in res.results], axis=0)[:N]
    proj = np.concatenate([r["proj_out"] for r in res.results], axis=0)[:N]
    return (z, proj)
